# revision 44
# baseline (speedup 1.0000x reference)
"""Trainium2 Bass kernel for nn_DecRnn: single-step attention-GRU decoder.

Strategy (8 NeuronCores):
  - Attention is data-parallel over batch (8 rows/core).
  - GRU is sharded over hidden dim (64 h-rows/core).
  - Vocab projection W_out/b_out sharded over vocab (6283 rows/core).
  - Two small AllGathers stitch it together:
      AG1: weighted^T shards  [1024, 8]  -> [1024, 64]
      AG2: h_new^T shards     [64, 64]   -> [512, 64]
  - All biases are folded into matmuls via ones-rows.
  - The big W_out matmul is split into 3 accumulation phases
    (bias+emb / weighted / h_new) so PE work tracks data availability
    and overlaps the W_out DMA stream.

Self-contained: hardcodes shapes/sharding; imports only installed libs.
"""

import numpy as np
from contextlib import ExitStack

import concourse.bass as bass
import concourse.mybir as mybir
import concourse.tile as tile
from concourse.tile import add_dep_helper
from concourse.bass_utils import run_bass_kernel_spmd
from concourse.masks import make_identity
from concourse.vector_clock import ScopedClock, VectorClock

# ---------------------------------------------------------------- constants
B, H, E, V, S = 64, 512, 300, 50257, 128
NC = 8           # cores
BC = B // NC     # batch rows per core (8)
HC = H // NC     # GRU h-rows per core (64)
VC = 6283        # vocab rows per core (8*6283 = 50264 >= V)
NW = 13          # vocab windows (13*484 = 6292 >= 6283)
WV = 484         # vocab window width
NK = 16          # x_cat k-chunks: [ones, emb x3, wtd x8, hnew x4]
F32 = mybir.dt.float32
I32 = mybir.dt.int32

# dtype knobs: "f32", "f32r", "f16", "bf16"
import os
DT_ATT = os.environ.get("KERN_DT_ATT", "f16")  # attention matmuls
DT_BIG = os.environ.get("KERN_DT_BIG", "f16")  # vocab projection matmuls

_STORE = {"f32": mybir.dt.float32, "f32r": mybir.dt.float32r,
          "f16": mybir.dt.float16, "bf16": mybir.dt.bfloat16}
_NP = {"f32": np.float32, "f32r": np.float32, "f16": np.float16}
try:
    import ml_dtypes
    _NP["bf16"] = ml_dtypes.bfloat16
except Exception:
    pass


def _mm(ap, mode):
    return ap


# ------------------------------------------------------ wait-split post-pass
# This container's walrus rejects more than MAX_WAITS sync waits on a single
# instruction. Move excess waits onto same-engine NOPs placed immediately
# before the instruction (semantics preserved: waits run before the inst).
MAX_WAITS = 1


def _split_excess_waits(nc, max_waits=MAX_WAITS):
    uid = 0
    for fn in nc.m.functions:
        for bb in fn.blocks:
            insts = bb.instructions
            out = []
            changed = False
            for inst in insts:
                si = inst.sync_info
                waits = list(si.on_wait) if si is not None else []
                if len(waits) > max_waits:
                    changed = True
                    excess = waits[:-max_waits]
                    keep = waits[-max_waits:]
                    for j in range(0, len(excess), max_waits):
                        grp = excess[j:j + max_waits]
                        nop = mybir.InstNoOp(
                            name=f"wsplit-{uid}",
                            engine=inst.engine,
                            ins=[], outs=[],
                            sync_info=mybir.SyncInfo(on_wait=grp,
                                                     on_update=[]))
                        uid += 1
                        out.append(nop)
                    inst.sync_info = mybir.SyncInfo(
                        on_wait=keep, on_update=list(si.on_update))
                out.append(inst)
            if changed:
                bb.instructions = out


# ---------------------------------------------------------------- builder
def build_nc():
    nc = bass.Bass("TRN2", target_bir_lowering=False, debug=False,
                   num_devices=NC)
    dA = _STORE[DT_ATT]
    dB = _STORE[DT_BIG]
    # fp32r is only legal/profitable for wide moving operands (N >= 256);
    # small-N matmuls (scores, weighted, hid_proj) use plain f32 instead.
    dAs = dA if DT_ATT in ("f16", "bf16") else F32

    # ---- I/O -------------------------------------------------------------
    enc_nat = nc.dram_tensor("enc_nat", [BC, S, 2 * H], dAs, kind="ExternalInput")
    encT_q = nc.dram_tensor("encT_q", [2, 8, 128, 512], dA, kind="ExternalInput")
    w2T_t = nc.dram_tensor("w2T_t", [8, 4, 128, 128], dA, kind="ExternalInput")
    w1T_t = nc.dram_tensor("w1T_t", [5, 4, 128, 128], dAs, kind="ExternalInput")
    hidT_aug = nc.dram_tensor("hidT_aug", [5, 128, BC], dAs, kind="ExternalInput")
    vT_t = nc.dram_tensor("vT_t", [4, 128, 1], dAs, kind="ExternalInput")
    mask_sh = nc.dram_tensor("mask_sh", [BC, S], I32, kind="ExternalInput")
    ids_in = nc.dram_tensor("ids_in", [B, 1], I32, kind="ExternalInput")
    emb_tab = nc.dram_tensor("emb_tab", [V, E], F32, kind="ExternalInput")
    F16 = mybir.dt.float16
    hid_g = nc.dram_tensor("hid_g", [4, 128, B], F16, kind="ExternalInput")
    hid_sh = nc.dram_tensor("hid_sh", [HC, B], F32, kind="ExternalInput")
    w_rz = nc.dram_tensor("w_rz", [16, 128, 128], F16, kind="ExternalInput")
    w_nx = nc.dram_tensor("w_nx", [12, 128, HC], F16, kind="ExternalInput")
    w_nh = nc.dram_tensor("w_nh", [5, 128, HC], F16, kind="ExternalInput")
    woT = nc.dram_tensor("woT", [NW, NK, 128, WV], dB, kind="ExternalInput")

    pred_out = nc.dram_tensor("pred_out", [B, NW * WV], F32, kind="ExternalOutput")
    hnewT_out = nc.dram_tensor("hnewT_out", [HC, B], F32, kind="ExternalOutput")
    a_out = nc.dram_tensor("a_out", [BC, S], F32, kind="ExternalOutput")

    with tile.TileContext(nc, num_cores=NC) as tc:
        with (
            tc.tile_pool(name="const", bufs=1) as cp,
            tc.tile_pool(name="wo", bufs=(19 if DT_BIG in ("f16", "bf16") else 4)) as wop,
            tc.tile_pool(name="pp", bufs=3, space="PSUM") as pp,
            tc.tile_pool(name="pe", bufs=2, space="PSUM") as pe_pool,
            tc.tile_pool(name="ps", bufs=3, space="PSUM") as ps,
            tc.tile_pool(name="dram", bufs=1, space="DRAM") as dp,
        ):
            # ---- constants ----------------------------------------------
            ident = cp.tile([128, 128], F32, name="ident", tag="ident")
            make_identity(nc, ident[:])
            ones_f32 = cp.tile([128, B], F32, name="ones_f32", tag="ones_f32")
            nc.gpsimd.memset(ones_f32[:], 0.0)
            nc.gpsimd.memset(ones_f32[0:1, :], 1.0)
            ones_d = cp.tile([128, B], dB, name="ones_d", tag="ones_d")
            nc.vector.tensor_copy(ones_d[:], ones_f32[:])

            # attention-phase tiles live in a scoped pool that closes after
            # the AG1 send, freeing ~57KB/partition for a second wo pool.
            _ap_es = ExitStack()
            ap = _ap_es.enter_context(tc.tile_pool(name="att", bufs=1))

            # ---- batched input loads (one DMA per tensor) ---------------
            ids_sb = cp.tile([B, 1], I32, name="ids_sb", tag="ids_sb")
            nc.sync.dma_start(ids_sb[:], ids_in[:])
            mask_sb = cp.tile([BC, S], I32, name="mask_sb", tag="mask_sb")
            nc.sync.dma_start(mask_sb[:], mask_sh[:])
            hida = ap.tile([128, 5 * BC], dAs, name="hida", tag="hida")
            nc.sync.dma_start(hida[:].rearrange("p (g b) -> p g b", g=5),
                              hidT_aug[:].rearrange("g p b -> p g b"))
            vTall = ap.tile([128, 4], dAs, name="vTall", tag="vTall")
            nc.sync.dma_start(vTall[:].rearrange("p (h o) -> p h o", h=4),
                              vT_t[:].rearrange("h p o -> p h o"))
            w1all = ap.tile([128, 20 * 128], dAs, name="w1all", tag="w1all")
            nc.sync.dma_start(w1all[:].rearrange("p (g h m) -> p g h m", g=5, h=4),
                              w1T_t[:].rearrange("g h p m -> p g h m"))
            w2all = ap.tile([128, 32 * 128], dA, name="w2all", tag="w2all")
            nc.sync.dma_start(w2all[:].rearrange("p (f h m) -> p f h m", f=8, h=4),
                              w2T_t[:].rearrange("f h p m -> p f h m"))
            encq_sb = []
            for q in range(2):
                t = ap.tile([128, 8 * 512], dA, name=f"eq{q}", tag=f"eq{q}")
                nc.sync.dma_start(t[:].rearrange("p (f j) -> p f j", f=8),
                                  encT_q[q].rearrange("f p j -> p f j"))
                encq_sb.append(t)
            encn = ap.tile([128, BC * 1024], dAs, name="encn", tag="encn")
            nc.sync.dma_start(encn[:].rearrange("s (b f) -> s b f", b=BC),
                              enc_nat[:].rearrange("b s f -> s b f"))
            hidg_sb = cp.tile([128, 4 * B], F16, name="hidg_sb", tag="hidg_sb")
            nc.sync.dma_start(hidg_sb[:].rearrange("p (j b) -> p j b", j=4),
                              hid_g[:].rearrange("j p b -> p j b"))
            hidsh_sb = cp.tile([HC, B], F32, name="hidsh_sb", tag="hidsh_sb")
            nc.sync.dma_start(hidsh_sb[:], hid_sh[:])
            wrzall = cp.tile([128, 16 * 128], F16, name="wrzall", tag="wrzall")
            nc.sync.dma_start(wrzall[:].rearrange("p (k m) -> p k m", k=16),
                              w_rz[:].rearrange("k p m -> p k m"))
            wnxall = cp.tile([128, 12 * HC], F16, name="wnxall", tag="wnxall")
            nc.sync.dma_start(wnxall[:].rearrange("p (k m) -> p k m", k=12),
                              w_nx[:].rearrange("k p m -> p k m"))
            wnhall = cp.tile([128, 5 * HC], F16, name="wnhall", tag="wnhall")
            nc.sync.dma_start(wnhall[:].rearrange("p (k m) -> p k m", k=5),
                              w_nh[:].rearrange("k p m -> p k m"))

            def w2_ap(fc, hc):
                o = (fc * 4 + hc) * 128
                return w2all[:, o:o + 128]

            def w1_ap(gc_i, hc):
                o = (gc_i * 4 + hc) * 128
                return w1all[:, o:o + 128]

            # ---- embedding gather + transpose ---------------------------
            rows_sb = ap.tile([B, E], F32, name="rows_sb", tag="rows_sb")
            nc.gpsimd.indirect_dma_start(
                out=rows_sb[:], out_offset=None, in_=emb_tab[:],
                in_offset=bass.IndirectOffsetOnAxis(ap=ids_sb[:, :1], axis=0))
            emb_f32 = cp.tile([128, 3 * B], F32, name="emb_f32", tag="emb_f32")
            emb_d = cp.tile([128, 3 * B], dB, name="emb_d", tag="emb_d")
            nc.gpsimd.memset(emb_f32[:], 0.0)
            tpall = ps.tile([128, 3 * B], F32, name="tpall", tag="ps")
            for kc in range(3):
                lo = kc * 128
                w = min(lo + 128, E) - lo
                nc.tensor.transpose(tpall[:w, kc * B:(kc + 1) * B],
                                    rows_sb[:, lo:lo + w], ident[:B, :B])
                nc.any.tensor_copy(emb_f32[:w, kc * B:(kc + 1) * B],
                                   tpall[:w, kc * B:(kc + 1) * B])
            nc.vector.tensor_copy(emb_d[:], emb_f32[:])
            F16L = mybir.dt.float16
            emb_16 = cp.tile([128, 3 * B], F16L, name="emb_16", tag="emb_16")
            nc.vector.tensor_copy(emb_16[:], emb_f32[:])
            ones_16 = cp.tile([128, B], F16L, name="ones_16", tag="ones_16")
            nc.vector.tensor_copy(ones_16[:], ones_f32[:])


            # ---- hid_proj (+b_attn): bias cols for energy tanh ----------
            phpall = ps.tile([128, 4 * BC], F32, name="phpall", tag="ps")
            for hc in range(4):
                for gc_i in range(5):
                    nc.tensor.matmul(phpall[:, hc * BC:(hc + 1) * BC],
                                     w1_ap(gc_i, hc),
                                     hida[:, gc_i * BC:(gc_i + 1) * BC],
                                     start=(gc_i == 0), stop=(gc_i == 4))
            hpall = ap.tile([128, 4 * BC], F32, name="hpall", tag="hpall")
            nc.any.tensor_copy(hpall[:], phpall[:])

            # ---- energy + tanh ------------------------------------------
            tanhE = {}
            for q in range(2):
                for hc in range(4):
                    pe = pe_pool.tile([128, 512], F32, name=f"pe{q}_{hc}",
                                      tag="pe")
                    for fc in range(8):
                        nc.tensor.matmul(
                            pe[:], w2_ap(fc, hc),
                            encq_sb[q][:, fc * 512:(fc + 1) * 512],
                            start=(fc == 0), stop=(fc == 7))
                    te = ap.tile([128, 512], dAs, name=f"te{q}_{hc}",
                                 tag=f"te{q}_{hc}")
                    for bq in range(4):
                        bl = 4 * q + bq
                        nc.scalar.activation(
                            te[:, bq * 128:(bq + 1) * 128],
                            pe[:, bq * 128:(bq + 1) * 128],
                            mybir.ActivationFunctionType.Tanh,
                            bias=hpall[:, hc * BC + bl:hc * BC + bl + 1])
                    tanhE[(q, hc)] = te

            # ---- scores: v dot tanhE ------------------------------------
            psT = ps.tile([128, BC], F32, name="psT", tag="ps")
            for bl in range(BC):
                q, bq = bl // 4, bl % 4
                for hc in range(4):
                    nc.tensor.matmul(
                        psT[:, bl:bl + 1],
                        tanhE[(q, hc)][:, bq * 128:(bq + 1) * 128],
                        vTall[:, hc:hc + 1],
                        start=(hc == 0), stop=(hc == 3))
            sT_sb = ap.tile([128, BC], F32, name="sT_sb", tag="sT_sb")
            nc.any.tensor_copy(sT_sb[:], psT[:])
            ts_p = ps.tile([BC, S], F32, name="ts_p", tag="ps")
            nc.tensor.transpose(ts_p[:], sT_sb[:], ident[:])
            scores = ap.tile([BC, S], F32, name="scores", tag="scores")
            nc.any.tensor_copy(scores[:], ts_p[:])

            # ---- mask + softmax (rows = local batch) --------------------
            m_f = ap.tile([BC, S], F32, name="m_f", tag="m_f")
            nc.vector.tensor_copy(m_f[:], mask_sb[:])
            negb = ap.tile([BC, S], F32, name="negb", tag="negb")
            nc.scalar.activation(negb[:], m_f[:],
                                 mybir.ActivationFunctionType.Copy,
                                 bias=-1e10, scale=1e10)
            t1 = ap.tile([BC, S], F32, name="t1", tag="t1")
            nc.vector.tensor_tensor(out=t1[:], in0=scores[:], in1=m_f[:],
                                    op=mybir.AluOpType.mult)
            masked = ap.tile([BC, S], F32, name="masked", tag="masked")
            nc.vector.tensor_tensor(out=masked[:], in0=t1[:], in1=negb[:],
                                    op=mybir.AluOpType.add)
            mx = ap.tile([BC, 1], F32, name="mx", tag="mx")
            nc.vector.reduce_max(mx[:], masked[:], axis=mybir.AxisListType.X)
            nmx = ap.tile([BC, 1], F32, name="nmx", tag="nmx")
            nc.vector.tensor_scalar_mul(nmx[:], mx[:], -1.0)
            e_sb = ap.tile([BC, S], F32, name="e_sb", tag="e_sb")
            nc.scalar.activation(e_sb[:], masked[:],
                                 mybir.ActivationFunctionType.Exp,
                                 bias=nmx[:, 0:1])
            ssum = ap.tile([BC, 1], F32, name="ssum", tag="ssum")
            nc.vector.reduce_sum(ssum[:], e_sb[:], axis=mybir.AxisListType.X)
            rinv = ap.tile([BC, 1], F32, name="rinv", tag="rinv")
            nc.vector.reciprocal(rinv[:], ssum[:])
            a_sb = ap.tile([BC, S], F32, name="a_sb", tag="a_sb")
            nc.vector.tensor_tensor(out=a_sb[:], in0=e_sb[:],
                                    in1=rinv[:].to_broadcast([BC, S]),
                                    op=mybir.AluOpType.mult)
            nc.scalar.dma_start(a_out[:], a_sb[:])

            # aT for the weighted matmul
            ta_p = ps.tile([128, BC], F32, name="ta_p", tag="ps")
            nc.tensor.transpose(ta_p[:], a_sb[:], ident[:BC, :BC])
            aT_d = ap.tile([128, BC], dAs, name="aT_d", tag="aT_d")
            nc.any.tensor_copy(aT_d[:], ta_p[:])

            # ---- weighted^T columns (local batch) -----------------------
            wt_f = ap.tile([128, 8 * BC], F32, name="wt_f", tag="wt_f")
            pwall = ps.tile([128, 8 * BC], F32, name="pwall", tag="ps")
            for fc in range(8):
                for bl in range(BC):
                    nc.tensor.matmul(
                        pwall[:, fc * BC + bl:fc * BC + bl + 1],
                        encn[:, bl * 1024 + fc * 128:bl * 1024 + (fc + 1) * 128],
                        aT_d[:, bl:bl + 1],
                        start=True, stop=True)
            nc.any.tensor_copy(wt_f[:], pwall[:])

            # ---- AG1: weighted shards -> full batch ---------------------
            wt_16 = ap.tile([128, 8 * BC], F16L, name="wt_16", tag="wt_16")
            nc.vector.tensor_copy(wt_16[:], wt_f[:])
            wh_b1 = dp.tile([8, 128, BC], F16L, name="wh_b1", tag="wh_b1")
            nc.gpsimd.dma_start(
                wh_b1[:].rearrange("s p b -> p s b"),
                wt_16[:].rearrange("p (s b) -> p s b", s=8))
            wt_g1 = dp.tile([NC, 8, 128, BC], F16L, name="wt_g1", tag="wt_g1",
                            addr_space="Shared")
            nc.gpsimd.collective_compute(
                "AllGather", mybir.AluOpType.bypass,
                ins=[wh_b1.opt()], outs=[wt_g1.opt()],
                replica_groups=[list(range(NC))])

            _ap_es.close()
            _wo2_es = ExitStack()
            wop2 = _wo2_es.enter_context(tc.tile_pool(
                name="wo2", bufs=(14 if DT_BIG in ("f16", "bf16") else 4)))

            # ---- vocab phase P1: bias + embedding chunks ----------------
            def xk1_ap(k):
                if k == 0:
                    return ones_d[:]
                return emb_d[:, (k - 1) * B:k * B]

            predall = cp.tile([B, NW * WV], F32, name="predall",
                              tag="predall")
            predB = cp.tile([B, NW * WV], F32, name="predB", tag="predB")
            for w in range(NW):
                blk = wop.tile([128, 4 * WV], dB, name=f"wo1_{w}", tag="wo")
                nc.sync.dma_start(
                    blk[:].rearrange("p (k v) -> p k v", k=4),
                    woT[w, 0:4].rearrange("k p v -> p k v"))
                p1 = pp.tile([B, WV], F32, name=f"p1_{w}", tag="pp")
                for k in range(4):
                    nc.tensor.matmul(p1[:], xk1_ap(k),
                                     blk[:, k * WV:(k + 1) * WV],
                                     start=(k == 0), stop=(k == 3))
                nc.any.tensor_copy(predall[:, w * WV:(w + 1) * WV], p1[:])

            # ---- AG1 readback (f16) -------------------------------------
            xw_16 = cp.tile([128, 8 * B], F16L, name="xw_16", tag="xw_16")
            for sec in range(8):
                nc.scalar.dma_start(
                    xw_16[:, sec * B:(sec + 1) * B].rearrange(
                        "p (c b) -> p c b", c=NC),
                    wt_g1[:, sec, :, :].rearrange("c p b -> p c b"))
            if dB != F16L:
                xw_d = cp.tile([128, 8 * B], dB, name="xw_d", tag="xw_d")
                nc.vector.tensor_copy(xw_d[:], xw_16[:])
            else:
                xw_d = xw_16

            # ---- GRU (m-shard rows, full batch), f16 matmuls ------------
            def xh_g(k):
                # chunk order: emb x3, wtd x8, hid x4, ones
                if k < 3:
                    return emb_16[:, k * B:(k + 1) * B]
                if k < 11:
                    return xw_16[:, (k - 3) * B:(k - 2) * B]
                if k < 15:
                    return hidg_sb[:, (k - 11) * B:(k - 10) * B]
                return ones_16[:]

            pgrz = ps.tile([128, B], F32, name="pgrz", tag="ps")
            for k in range(16):
                nc.tensor.matmul(pgrz[:], wrzall[:, k * 128:(k + 1) * 128],
                                 xh_g(k), start=(k == 0), stop=(k == 15))
            rz_sb = cp.tile([128, B], F32, name="rz_sb", tag="rz_sb")
            nc.scalar.activation(rz_sb[:], pgrz[:],
                                 mybir.ActivationFunctionType.Sigmoid)
            pgnx = ps.tile([HC, B], F32, name="pgnx", tag="ps")
            for k in range(12):
                nc.tensor.matmul(pgnx[:], wnxall[:, k * HC:(k + 1) * HC],
                                 xh_g(k if k < 11 else 15),
                                 start=(k == 0), stop=(k == 11))
            pgnh = ps.tile([HC, B], F32, name="pgnh", tag="ps")
            for k in range(5):
                nc.tensor.matmul(pgnh[:], wnhall[:, k * HC:(k + 1) * HC],
                                 xh_g(11 + k), start=(k == 0), stop=(k == 4))
            z_sb = cp.tile([HC, B], F32, name="z_sb", tag="z_sb")
            nc.vector.tensor_copy(z_sb[:], rz_sb[HC:2 * HC, :])
            rgnh = cp.tile([HC, B], F32, name="rgnh", tag="rgnh")
            nc.vector.tensor_tensor(out=rgnh[:], in0=rz_sb[0:HC, :],
                                    in1=pgnh[:], op=mybir.AluOpType.mult)
            pre_n = cp.tile([HC, B], F32, name="pre_n", tag="pre_n")
            nc.vector.tensor_tensor(out=pre_n[:], in0=rgnh[:], in1=pgnx[:],
                                    op=mybir.AluOpType.add)
            n_sb = cp.tile([HC, B], F32, name="n_sb", tag="n_sb")
            nc.scalar.activation(n_sb[:], pre_n[:],
                                 mybir.ActivationFunctionType.Tanh)
            # h_new = n + z * (hid - n)
            hmn = cp.tile([HC, B], F32, name="hmn", tag="hmn")
            nc.vector.tensor_tensor(out=hmn[:], in0=hidsh_sb[:], in1=n_sb[:],
                                    op=mybir.AluOpType.subtract)
            zd = cp.tile([HC, B], F32, name="zd", tag="zd")
            nc.vector.tensor_tensor(out=zd[:], in0=z_sb[:], in1=hmn[:],
                                    op=mybir.AluOpType.mult)
            hn_16 = cp.tile([HC, B], F16L, name="hn_16", tag="hn_16")
            nc.vector.tensor_tensor(out=hn_16[:], in0=n_sb[:], in1=zd[:],
                                    op=mybir.AluOpType.add)
            hnT = cp.tile([HC, B], F32, name="hnT", tag="hnT")
            nc.vector.tensor_tensor(out=hnT[:], in0=n_sb[:], in1=zd[:],
                                    op=mybir.AluOpType.add)
            nc.scalar.dma_start(hnewT_out[:], hnT[:])

            # ---- AG2: h_new shards --------------------------------------
            wh_b2 = dp.tile([HC, B], F16L, name="wh_b2", tag="wh_b2")
            nc.gpsimd.dma_start(wh_b2[:], hn_16[:])
            hn_g2 = dp.tile([NC, HC, B], F16L, name="hn_g2", tag="hn_g2",
                            addr_space="Shared")
            nc.gpsimd.collective_compute(
                "AllGather", mybir.AluOpType.bypass,
                ins=[wh_b2.opt()], outs=[hn_g2.opt()],
                replica_groups=[list(range(NC))])

            # ---- vocab phase P2: weighted chunks ------------------------
            for w in range(NW):
                p2 = pp.tile([B, WV], F32, name=f"p2_{w}", tag="pp")
                for half in range(2):
                    pool2 = wop if half == 0 else wop2
                    blk = pool2.tile([128, 4 * WV], dB, name=f"wo2_{w}_{half}",
                                     tag=("wo" if half == 0 else "wo2b"))
                    nc.sync.dma_start(
                        blk[:].rearrange("p (k v) -> p k v", k=4),
                        woT[w, 4 + 4 * half:8 + 4 * half].rearrange(
                            "k p v -> p k v"))
                    for j in range(4):
                        kk = 4 * half + j
                        nc.tensor.matmul(p2[:], xw_d[:, kk * B:(kk + 1) * B],
                                         blk[:, j * WV:(j + 1) * WV],
                                         start=(kk == 0), stop=(kk == 7))
                nc.any.tensor_copy(predB[:, w * WV:(w + 1) * WV], p2[:])

            # merge P2 partial during the AG2 window
            for w in range(NW):
                nc.vector.tensor_tensor(
                    out=predall[:, w * WV:(w + 1) * WV],
                    in0=predall[:, w * WV:(w + 1) * WV],
                    in1=predB[:, w * WV:(w + 1) * WV],
                    op=mybir.AluOpType.add)

            # ---- AG2 readback (f16) -------------------------------------
            xh_16 = cp.tile([128, 4 * B], F16L, name="xh_16", tag="xh_16")
            for j in range(4):
                nc.scalar.dma_start(
                    xh_16[:, j * B:(j + 1) * B],
                    hn_g2[2 * j:2 * j + 2].rearrange("c h b -> (c h) b"))
            if dB != F16L:
                xh_d = cp.tile([128, 4 * B], dB, name="xh_d", tag="xh_d")
                nc.vector.tensor_copy(xh_d[:], xh_16[:])
            else:
                xh_d = xh_16

            # ---- vocab phase P3: hnew chunks + output -------------------
            for w in range(NW):
                blk = wop2.tile([128, 4 * WV], dB, name=f"wo3_{w}", tag="wo2b")
                nc.sync.dma_start(
                    blk[:].rearrange("p (k v) -> p k v", k=4),
                    woT[w, 12:16].rearrange("k p v -> p k v"))
                p3 = pp.tile([B, WV], F32, name=f"p3_{w}", tag="pp")
                for j in range(4):
                    nc.tensor.matmul(p3[:], xh_d[:, j * B:(j + 1) * B],
                                     blk[:, j * WV:(j + 1) * WV],
                                     start=(j == 0), stop=(j == 3))
                nc.vector.tensor_tensor(
                    out=predall[:, w * WV:(w + 1) * WV],
                    in0=predall[:, w * WV:(w + 1) * WV],
                    in1=p3[:], op=mybir.AluOpType.add)
                nc.scalar.dma_start(pred_out[:, w * WV:(w + 1) * WV],
                                    predall[:, w * WV:(w + 1) * WV])
            _wo2_es.close()

    _split_excess_waits(nc)
    return nc


_NC_CACHE = {}


def _get_nc():
    key = (DT_ATT, DT_BIG)
    if key not in _NC_CACHE:
        _NC_CACHE[key] = build_nc()
    return _NC_CACHE[key]


# ------------------------------------------------------------- host prep
def _prep_core(c, input_ids, hidden, encoder_outputs, mask,
               emb_table, W_attn, b_attn, v_w, W_ih, W_hh, b_ih, b_hh,
               W_out, b_out):
    npA = _NP[DT_ATT]
    npB = _NP[DT_BIG]
    brows = slice(BC * c, BC * (c + 1))
    hr0 = HC * c
    vlo = VC * c
    vhi = min(VC * (c + 1), V)
    nv = vhi - vlo

    enc_b = encoder_outputs[:, brows, :]                    # [S, BC, 2H]
    enc_nat = np.ascontiguousarray(
        enc_b.transpose(1, 0, 2)).astype(npA)               # [BC, S, 2H]
    e = enc_b.transpose(1, 2, 0).reshape(2, 4, 8, 128, 128)  # [q,bq,fc,p,s]
    encT_q = np.ascontiguousarray(
        e.transpose(0, 2, 3, 1, 4).reshape(2, 8, 128, 512)).astype(npA)

    w2 = W_attn[:, H:].T.reshape(8, 128, 4, 128)            # [fc,p,hc,m]
    w2T_t = np.ascontiguousarray(w2.transpose(0, 2, 1, 3)).astype(npA)
    w1aug = np.zeros((640, H), np.float32)
    w1aug[:H] = W_attn[:, :H].T
    w1aug[H] = b_attn
    w1T_t = np.ascontiguousarray(
        w1aug.reshape(5, 128, 4, 128).transpose(0, 2, 1, 3)).astype(npA)
    haug = np.zeros((640, BC), np.float32)
    haug[:H] = hidden[brows].T
    haug[H] = 1.0
    hidT_aug = haug.reshape(5, 128, BC).astype(npA)
    vT_t = v_w[0].reshape(4, 128, 1).astype(npA)

    # GRU m-shard weights (rows for this core's 64 h), f16
    f16 = np.float16
    idx_rz = np.r_[hr0:hr0 + HC, H + hr0:H + hr0 + HC]
    idx_n = np.arange(2 * H + hr0, 2 * H + hr0 + HC)
    Wih_rz = W_ih[idx_rz]                                   # [128, 1324]
    Whh_rz = W_hh[idx_rz]                                   # [128, 512]
    wrz = np.zeros((16, 128, 128), f16)
    tmp_e = np.zeros((384, 128), np.float32)
    tmp_e[:E] = Wih_rz[:, :E].T
    wrz[0:3] = tmp_e.reshape(3, 128, 128).astype(f16)
    wrz[3:11] = Wih_rz[:, E:].T.reshape(8, 128, 128).astype(f16)
    wrz[11:15] = Whh_rz.T.reshape(4, 128, 128).astype(f16)
    wrz[15, 0, :] = (b_ih + b_hh)[idx_rz].astype(f16)
    Wih_n = W_ih[idx_n]                                     # [64, 1324]
    Whh_n = W_hh[idx_n]                                     # [64, 512]
    wnx = np.zeros((12, 128, HC), f16)
    tmp_e = np.zeros((384, HC), np.float32)
    tmp_e[:E] = Wih_n[:, :E].T
    wnx[0:3] = tmp_e.reshape(3, 128, HC).astype(f16)
    wnx[3:11] = Wih_n[:, E:].T.reshape(8, 128, HC).astype(f16)
    wnx[11, 0, :] = b_ih[idx_n].astype(f16)
    wnh = np.zeros((5, 128, HC), f16)
    wnh[0:4] = Whh_n.T.reshape(4, 128, HC).astype(f16)
    wnh[4, 0, :] = b_hh[idx_n].astype(f16)

    # vocab shard, tiled [NW, NK, 128, WV]
    tmp = np.zeros((2048, NW * WV), np.float32)
    Wc = W_out[vlo:vhi]                                     # [nv, 1836]
    tmp[0, :nv] = b_out[vlo:vhi]
    tmp[128:128 + E, :nv] = Wc[:, 3 * H:].T                 # embedded
    tmp[512:1536, :nv] = Wc[:, H:3 * H].T                   # weighted
    tmp[1536:2048, :nv] = Wc[:, :H].T                       # h_new
    woT = np.ascontiguousarray(
        tmp.reshape(16, 128, NW, WV).transpose(2, 0, 1, 3)).astype(npB)

    return {
        "enc_nat": enc_nat, "encT_q": encT_q, "w2T_t": w2T_t,
        "w1T_t": w1T_t, "hidT_aug": hidT_aug, "vT_t": vT_t,
        "mask_sh": np.ascontiguousarray(mask[brows]).astype(np.int32),
        "ids_in": input_ids.astype(np.int32).reshape(B, 1),
        "emb_tab": emb_table.astype(np.float32),
        "hid_g": np.ascontiguousarray(
            hidden.T.reshape(4, 128, B)).astype(np.float16),
        "hid_sh": np.ascontiguousarray(
            hidden.T[hr0:hr0 + HC]).astype(np.float32),
        "w_rz": wrz, "w_nx": wnx, "w_nh": wnh, "woT": woT,
    }


def kernel(input_ids, hidden, encoder_outputs, mask, emb_table,
           W_attn, b_attn, v_w, W_ih, W_hh, b_ih, b_hh, W_out, b_out,
           _trace=False, _tmpdir=None):
    args = [np.asarray(x) for x in
            (input_ids, hidden, encoder_outputs, mask, emb_table, W_attn,
             b_attn, v_w, W_ih, W_hh, b_ih, b_hh, W_out, b_out)]
    in_maps = [_prep_core(c, *args) for c in range(NC)]

    nc = _get_nc()
    res = run_bass_kernel_spmd(nc, in_maps, core_ids=list(range(NC)),
                               trace=_trace, tmpdir=_tmpdir)

    pred_full = np.zeros((B, NC * VC), np.float32)
    hT = np.zeros((H, B), np.float32)
    a_full = np.zeros((B, S), np.float32)
    for c in range(NC):
        r = res.results[c]
        pred_full[:, VC * c:VC * (c + 1)] = r["pred_out"][:, :VC]
        hT[HC * c:HC * (c + 1)] = r["hnewT_out"]
        a_full[BC * c:BC * (c + 1)] = r["a_out"]
    prediction = pred_full[:, :V]
    h_new = np.ascontiguousarray(hT.T)
    kernel._last_exec_time_ns = res.exec_time_ns
    return prediction, h_new, a_full
